# revision 1
# baseline (speedup 1.0000x reference)
"""Batched GAT kernel for 8 Trainium2 NeuronCores.

Math: out[b,i,:] = softmax_j(mask(leakyrelu(s_i+t_j))) @ h  per head, concat heads.

Decomposition (avoids exp on the [i,j] pairwise data entirely):
  exp(lrelu(e)) = max(u_i*v_j, u'_i*v'_j)   u=exp(s), v=exp(t), u'=exp(.2s), v'=exp(.2t)
                = G*(u v) + (1-G)*(u' v'),  G = 1[e>=0]
  p = m * that, m = 1[adj>0.5]
  num_f = u_i*(Gm @ v.h)_f + u'_i*((M @ v'.h)_f - (Gm @ v'.h)_f);  den analogous.

Per core (c = 0..7): b = c//2, rows i in [ (c%2)*1024, +1024 ).
Gm is built per (head, j-tile) on DVE: tensor_scalar (s_bcast >= -t_col) at 4x,
then tensor_tensor * mT at 2x (a few j-tiles offloaded to GPSIMD).
PE consumes Gm/M as fp16 rhs against fp16 value-packs; mask transpose is done
on PE (128x128 fp16 transpose-matmuls), NOT DMA-xbar (HWDGE per-instruction
overhead serializes, and cross-queue xbar transpose/copy races corrupt data).
"""
import os
import sys
import numpy as np

for _p in ("/opt/trn_rl_repo",):
    if _p not in sys.path:
        sys.path.insert(0, _p)

B, N, D, H, F = 4, 2048, 128, 4, 32
HF = H * F           # 128
IR = 1024            # i-rows per core
NJT = N // 128       # 16 j-tiles
NCORES = 8

_CACHE = {}


def build_nc(reps=1):
    import concourse.bacc as bacc
    import concourse.tile as tile
    from concourse import mybir

    f32, f16 = mybir.dt.float32, mybir.dt.float16
    Alu = mybir.AluOpType
    Act = mybir.ActivationFunctionType

    nc = bacc.Bacc(None, target_bir_lowering=False)

    xT_d   = nc.dram_tensor("xT",   [D, N],    f32, kind="ExternalInput")
    xiT_d  = nc.dram_tensor("xiT",  [D, IR],   f32, kind="ExternalInput")
    adj_d  = nc.dram_tensor("adjS", [IR, N],   f32, kind="ExternalInput")
    Wf_d   = nc.dram_tensor("Wf",   [D, HF],   f32, kind="ExternalInput")
    aS_d   = nc.dram_tensor("aS",   [HF, H],   f32, kind="ExternalInput")
    aD_d   = nc.dram_tensor("aD",   [HF, H],   f32, kind="ExternalInput")
    bias_d = nc.dram_tensor("biasC", [HF, 1],  f32, kind="ExternalInput")
    out_d  = nc.dram_tensor("out",  [IR, HF],  f32, kind="ExternalOutput")

    # host constants
    S4 = np.zeros((128, 4), np.float32)
    B4 = np.zeros((4, 128), np.float32)
    for h in range(H):
        S4[h * 32, h] = 1.0
        B4[h, h * 32:(h + 1) * 32] = 1.0
    EY = np.zeros((4, 4 * 128), np.float16)
    for h in range(H):
        EY[h, h * 128:(h + 1) * 128] = 1.0
    S4_d = nc.inline_tensor(S4, "S4c")
    B4_d = nc.inline_tensor(B4, "B4c")
    EY_d = nc.inline_tensor(EY, "EYc")
    ID_d = nc.inline_tensor(np.eye(128, dtype=np.float32), "identc")
    ID16_d = nc.inline_tensor(np.eye(128, dtype=np.float16), "ident16c")

    with tile.TileContext(nc) as tc:
        cst_ctx = tc.tile_pool(name="cst", bufs=1)
        cst = cst_ctx.__enter__()
        try:
            # ---------------- persistent tiles ----------------
            prep_ctx = tc.tile_pool(name="prep", bufs=1)
            prep = prep_ctx.__enter__()
            xT   = prep.tile([D, N], f32)
            xiT  = prep.tile([D, IR], f32)
            hT   = prep.tile([HF, N], f32)
            hiT  = prep.tile([HF, IR], f32)
            h_sb = prep.tile([128, NJT, HF], f32)

            Wf   = cst.tile([D, HF], f32)
            aS   = cst.tile([HF, H], f32)
            aD   = cst.tile([HF, H], f32)
            biasC = cst.tile([HF, 1], f32)
            s4c  = cst.tile([128, 4], f32)
            b4c  = cst.tile([4, 128], f32)
            eyc  = cst.tile([4, 4 * 128], f16)
            idc  = cst.tile([128, 128], f32)
            idc16 = cst.tile([128, 128], f16)

            nc.sync.dma_start(xT[:], xT_d[:])
            nc.sync.dma_start(xiT[:], xiT_d[:])
            nc.sync.dma_start(Wf[:], Wf_d[:])
            nc.sync.dma_start(aS[:], aS_d[:])
            nc.sync.dma_start(aD[:], aD_d[:])
            nc.sync.dma_start(biasC[:], bias_d[:])
            nc.sync.dma_start(s4c[:], S4_d[:])
            nc.sync.dma_start(b4c[:], B4_d[:])
            nc.sync.dma_start(eyc[:], EY_d[:])
            nc.sync.dma_start(idc[:], ID_d[:])
            nc.sync.dma_start(idc16[:], ID16_d[:])

            sZ4   = cst.tile([4, IR], f32)
            u1r4  = cst.tile([4, IR], f32)
            u02r4 = cst.tile([4, IR], f32)
            s16_4 = cst.tile([4, IR], f16)
            negt  = cst.tile([128, NJT, H], f32)
            v1c   = cst.tile([128, NJT, H], f32)
            v02c  = cst.tile([128, NJT, H], f32)
            sb16  = cst.tile([128, H, IR], f16)
            u1b   = cst.tile([128, IR], f32)
            u02b  = cst.tile([128, IR], f32)
            Gpack = cst.tile([128, NJT, H, 128], f16)
            Mpack = cst.tile([128, NJT, HF], f16)
            MdenP = cst.tile([128, NJT, HF], f16)
            mT_all = cst.tile([128, NJT, IR], f16)

            As_all = cst.tile([128, IR], f32)
            Cs_all = cst.tile([128, IR], f32)
            DenA   = cst.tile([128, IR], f32)
            DenC   = cst.tile([128, IR], f32)
            Ms_sb  = cst.tile([128, IR], f32)
            Mds_sb = cst.tile([128, IR], f32)
            t1   = cst.tile([128, IR], f32)
            t2   = cst.tile([128, IR], f32)
            den4 = cst.tile([4, IR], f32)
            rd4  = cst.tile([4, IR], f32)
            rdb  = cst.tile([128, IR], f32)
            outT = cst.tile([128, IR], f32)
            out_sb = cst.tile([128, 8, HF], f32)

            adj_r = adj_d[:].rearrange("(s p) j -> p s j", p=128)

            # ---------------- phase emitters ----------------
            def emit_mask():
                """adj -> binarize -> PE transpose -> mT_all [j, jt, i] fp16."""
                with tc.tile_pool(name="adjp", bufs=3) as adjp, \
                     tc.tile_pool(name="mip", bufs=2) as mip, \
                     tc.tile_pool(name="mtp", bufs=4, space="PSUM") as mtp:
                    for blk in range(8):
                        at = adjp.tile([128, 8, 256], f32, tag="adj")
                        nc.sync.dma_start(at[:], adj_r[:, :, blk * 256:(blk + 1) * 256])
                        mi = mip.tile([128, 8, 256], f16, tag="mi")
                        beng = nc.gpsimd if blk % 2 == 1 else nc.vector
                        beng.tensor_scalar(mi[:], at[:], 0.5, None, op0=Alu.is_gt)
                        for q in range(2):
                            jt = blk * 2 + q
                            for half in range(2):
                                pt = mtp.tile([128, 512], f16, tag="mt")
                                for sub4 in range(4):
                                    sub = half * 4 + sub4
                                    nc.tensor.transpose(
                                        pt[:, sub4 * 128:(sub4 + 1) * 128],
                                        mi[:, sub, q * 128:(q + 1) * 128], idc16[:])
                                nc.scalar.copy(
                                    mT_all[:, jt, half * 512:(half + 1) * 512], pt[:])

            def emit_main():
                """G phase, mask matmuls, combine, output."""
                with tc.tile_pool(name="gp", bufs=5) as gp, \
                     tc.tile_pool(name="psg", bufs=2, space="PSUM") as psg, \
                     tc.tile_pool(name="psm", bufs=1, space="PSUM") as psm:
                    pM = psm.tile([128, IR], f32, tag="pm")
                    pMd = psm.tile([128, IR], f32, tag="pmd")
                    for h in range(H):
                        pg = psg.tile([128, IR], f32, tag="pg")
                        for jt in range(NJT):
                            onpool = jt in (5, 11)
                            gpre = gp.tile([128, IR], f16, tag="gpre")
                            (nc.gpsimd if onpool else nc.vector).tensor_scalar(
                                gpre[:], sb16[:, h, :], negt[:, jt, h:h + 1], None,
                                op0=Alu.is_ge)
                            g = gp.tile([128, IR], f16, tag="g")
                            (nc.gpsimd if onpool else nc.vector).tensor_tensor(
                                g[:], gpre[:], mT_all[:, jt, :], op=Alu.mult)
                            for k in range(2):
                                nc.tensor.matmul(
                                    pg[:, k * 512:(k + 1) * 512],
                                    Gpack[:, jt, h, :],
                                    g[:, k * 512:(k + 1) * 512],
                                    start=(jt == 0), stop=(jt == NJT - 1))
                            if h == 0:
                                for k in range(2):
                                    nc.tensor.matmul(
                                        pM[:, k * 512:(k + 1) * 512],
                                        Mpack[:, jt, :],
                                        mT_all[:, jt, k * 512:(k + 1) * 512],
                                        start=(jt == 0), stop=(jt == NJT - 1))
                                    nc.tensor.matmul(
                                        pMd[:, k * 512:(k + 1) * 512],
                                        MdenP[:, jt, :],
                                        mT_all[:, jt, k * 512:(k + 1) * 512],
                                        start=(jt == 0), stop=(jt == NJT - 1))
                        # drain this head's psum to SBUF (32-aligned blocks)
                        hs = slice(h * 32, (h + 1) * 32)
                        nc.scalar.copy(As_all[hs, :], pg[0:32, :])
                        nc.scalar.copy(DenA[hs, :], pg[32:64, :])
                        nc.scalar.copy(Cs_all[hs, :], pg[64:96, :])
                        nc.scalar.copy(DenC[hs, :], pg[96:128, :])
                    nc.scalar.copy(Ms_sb[:], pM[:])
                    nc.scalar.copy(Mds_sb[:], pMd[:])

                # combine: den chain on GPSIMD (parallel with num chain on DVE)
                dall = Mds_sb  # reuse
                num = Ms_sb    # reuse
                nc.gpsimd.tensor_tensor(t2[:], Mds_sb[:], DenC[:], op=Alu.add)
                nc.gpsimd.tensor_tensor(t2[:], t2[:], u02b[:], op=Alu.mult)
                nc.gpsimd.tensor_tensor(DenA[:], DenA[:], u1b[:], op=Alu.mult)
                nc.gpsimd.tensor_tensor(dall[:], t2[:], DenA[:], op=Alu.add)
                nc.vector.tensor_tensor(t1[:], Ms_sb[:], Cs_all[:], op=Alu.add)
                nc.vector.tensor_tensor(t1[:], t1[:], u02b[:], op=Alu.mult)
                nc.vector.tensor_tensor(As_all[:], As_all[:], u1b[:], op=Alu.mult)
                nc.vector.tensor_tensor(num[:], t1[:], As_all[:], op=Alu.add)

                with tc.tile_pool(name="fps", bufs=1, space="PSUM") as fps, \
                     tc.tile_pool(name="fpt", bufs=2, space="PSUM") as fpt:
                    pd = fps.tile([4, IR], f32, tag="pd")
                    for k in range(2):
                        nc.tensor.matmul(pd[:, k * 512:(k + 1) * 512], s4c[:],
                                         dall[:, k * 512:(k + 1) * 512],
                                         start=True, stop=True)
                    nc.scalar.copy(den4[:], pd[:])
                    nc.vector.reciprocal_approx_accurate(rd4[:], den4[:], t1[0:4, :])
                    prb = fps.tile([128, IR], f32, tag="prb")
                    for k in range(2):
                        nc.tensor.matmul(prb[:, k * 512:(k + 1) * 512], b4c[:],
                                         rd4[:, k * 512:(k + 1) * 512],
                                         start=True, stop=True)
                    nc.scalar.copy(rdb[:], prb[:])

                    nc.vector.tensor_tensor(outT[:], num[:], rdb[:], op=Alu.mult)
                    nc.vector.tensor_scalar(outT[:], outT[:], biasC[:, 0:1], None,
                                            op0=Alu.add)

                    for sub in range(8):
                        pt = fpt.tile([128, 128], f32, tag="pt")
                        nc.tensor.transpose(pt[:], outT[:, sub * 128:(sub + 1) * 128],
                                            idc[:])
                        nc.scalar.copy(out_sb[:, sub, :], pt[:])
                    nc.sync.dma_start(
                        out_d[:].rearrange("(s p) f -> p s f", p=128), out_sb[:])

            # ---------------- prep ----------------
            with tc.tile_pool(name="pp", bufs=4, space="PSUM") as pp:
                for k in range(2):
                    ps = pp.tile([HF, 512], f32, tag="pp")
                    nc.tensor.matmul(ps[:], Wf[:], xiT[:, k * 512:(k + 1) * 512],
                                     start=True, stop=True)
                    nc.vector.tensor_copy(hiT[:, k * 512:(k + 1) * 512], ps[:])
                for k in range(2):
                    ps = pp.tile([4, 512], f32, tag="pp")
                    nc.tensor.matmul(ps[:], aS[:], hiT[:, k * 512:(k + 1) * 512],
                                     start=True, stop=True)
                    nc.scalar.copy(sZ4[:, k * 512:(k + 1) * 512], ps[:])
                nc.vector.tensor_copy(s16_4[:], sZ4[:])
                nc.scalar.activation(u1r4[:], sZ4[:], Act.Exp)
                nc.scalar.activation(u02r4[:], sZ4[:], Act.Exp, scale=0.2)
                for k in range(4):
                    ps = pp.tile([HF, 512], f32, tag="pp")
                    nc.tensor.matmul(ps[:], Wf[:], xT[:, k * 512:(k + 1) * 512],
                                     start=True, stop=True)
                    nc.vector.tensor_copy(hT[:, k * 512:(k + 1) * 512], ps[:])
                for jt in range(NJT):
                    ps = pp.tile([128, H], f32, tag="pp")
                    nc.tensor.matmul(ps[:], hT[:, jt * 128:(jt + 1) * 128], aD[:],
                                     start=True, stop=True)
                    nc.scalar.mul(negt[:, jt, :], ps[:], -1.0)
                    nc.scalar.activation(v1c[:, jt, :], ps[:], Act.Exp)
                    nc.scalar.activation(v02c[:, jt, :], ps[:], Act.Exp, scale=0.2)
                for q4 in range(4):
                    ps = pp.tile([128, 512], f32, tag="pp")
                    for k4 in range(4):
                        jt = q4 * 4 + k4
                        nc.tensor.matmul(ps[:, k4 * 128:(k4 + 1) * 128],
                                         xT[:, jt * 128:(jt + 1) * 128], Wf[:],
                                         start=True, stop=True)
                    nc.scalar.copy(
                        h_sb[:].rearrange("p j f -> p (j f)")[:, q4 * 512:(q4 + 1) * 512],
                        ps[:])

                # broadcasts via PE
                for h in range(H):
                    for k in range(2):
                        ps = pp.tile([128, 512], f32, tag="pp")
                        nc.tensor.matmul(ps[:], eyc[:, h * 128:(h + 1) * 128],
                                         s16_4[:, k * 512:(k + 1) * 512],
                                         start=True, stop=True)
                        nc.scalar.copy(sb16[:, h, k * 512:(k + 1) * 512], ps[:])
                for src, dst in ((u1r4, u1b), (u02r4, u02b)):
                    for k in range(2):
                        ps = pp.tile([128, 512], f32, tag="pp")
                        nc.tensor.matmul(ps[:], b4c[:], src[:, k * 512:(k + 1) * 512],
                                         start=True, stop=True)
                        nc.scalar.copy(dst[:, k * 512:(k + 1) * 512], ps[:])

                # mask phase for rep 0 emitted here so binarize/transpose
                # interleave with the prep tail in every engine stream
                emit_mask()

                # ---------------- packs ----------------
                nc.gpsimd.memset(Gpack[:], 0.0)
                nc.gpsimd.memset(MdenP[:], 0.0)
                for h in range(H):
                    hsl = h_sb[:, :, h * 32:(h + 1) * 32]
                    nc.vector.tensor_tensor(
                        Mpack[:, :, h * 32:(h + 1) * 32], hsl,
                        v02c[:, :, h:h + 1].broadcast_to([128, NJT, 32]), op=Alu.mult)
                    nc.vector.tensor_tensor(
                        Gpack[:, :, h, 0:32], hsl,
                        v1c[:, :, h:h + 1].broadcast_to([128, NJT, 32]), op=Alu.mult)
                    nc.vector.tensor_scalar(
                        Gpack[:, :, h, 64:96], Mpack[:, :, h * 32:(h + 1) * 32],
                        -1.0, None, op0=Alu.mult)
                    nc.vector.tensor_copy(Gpack[:, :, h, 32:33], v1c[:, :, h:h + 1])
                    nc.vector.tensor_scalar(
                        Gpack[:, :, h, 96:97], v02c[:, :, h:h + 1], -1.0, None,
                        op0=Alu.mult)
                    nc.vector.tensor_copy(MdenP[:, :, h * 32:h * 32 + 1],
                                          v02c[:, :, h:h + 1])

            prep_ctx.__exit__(None, None, None)

            emit_main()
            for _rep in range(1, reps):
                emit_mask()
                emit_main()
        finally:
            cst_ctx.__exit__(None, None, None)

    nc.compile()
    return nc


def _prepare_in_maps(x, adj, W, a_src, a_dst, bias):
    x = np.ascontiguousarray(np.asarray(x, dtype=np.float32))
    adj = np.asarray(adj, dtype=np.float32)
    W = np.asarray(W, dtype=np.float32)
    a_src = np.asarray(a_src, dtype=np.float32)
    a_dst = np.asarray(a_dst, dtype=np.float32)
    bias = np.asarray(bias, dtype=np.float32)

    Wf = np.ascontiguousarray(W.reshape(D, HF))
    aS = np.zeros((HF, H), np.float32)
    aD = np.zeros((HF, H), np.float32)
    for h in range(H):
        aS[h * F:(h + 1) * F, h] = a_src[h]
        aD[h * F:(h + 1) * F, h] = a_dst[h]
    biasC = np.ascontiguousarray(bias.reshape(HF, 1))

    in_maps = []
    for c in range(NCORES):
        b, cc = c // 2, c % 2
        i0 = cc * IR
        in_maps.append({
            "xT": np.ascontiguousarray(x[b].T),
            "xiT": np.ascontiguousarray(x[b, i0:i0 + IR].T),
            "adjS": np.ascontiguousarray(adj[b, i0:i0 + IR, :]),
            "Wf": Wf,
            "aS": aS,
            "aD": aD,
            "biasC": biasC,
        })
    return in_maps


def run(inputs, trace=False, trace_cores=None):
    from concourse.bass_utils import run_bass_kernel_spmd
    if "nc" not in _CACHE:
        _CACHE["nc"] = build_nc()
    nc = _CACHE["nc"]
    in_maps = _prepare_in_maps(**inputs)
    kw = {}
    if trace:
        kw = dict(trace=True, trace_cores=trace_cores or [0])
    res = run_bass_kernel_spmd(nc, in_maps, list(range(NCORES)), **kw)
    out = np.zeros((B, N, HF), np.float32)
    for c in range(NCORES):
        b, cc = c // 2, c % 2
        out[b, cc * IR:(cc + 1) * IR, :] = res.results[c]["out"]
    return out, res


def kernel(**inputs):
    out, _ = run(inputs, trace=False)
    return out



# revision 8
# speedup vs baseline: 1.3498x; 1.3498x over previous
"""Batched GAT kernel for 8 Trainium2 NeuronCores.

Math: out[b,i,:] = softmax_j(mask(leakyrelu(s_i+t_j))) @ h  per head, concat heads.

Decomposition: exp(lrelu(e)) = max(u_i v_j, u'_i v'_j) with u=exp(s), v=exp(t),
u'=exp(.2s), v'=exp(.2t).  Dividing each row i by u'_i (cancels in softmax):
  p~_ij = m_ij * max(w_i v_j, v'_j),   w = exp(.8 s)
  out = (P~ @ [h|1]) -> num/den per head.  No G-indicator, no mask matmuls,
  no u-rescale combine: one matmul stream against a plain [h|1] fp16 pack.

Per core (c = 0..7): b = c//2, rows i in [ (c%2)*1024, +1024 ).
Per (head, j-tile): q = tensor_scalar(wb, *v_j, max v'_j) (DVE 4x) and
p~ = min(q, maskT) (DVE/GPSIMD 2x) where maskT in {0, BIG} comes from the
binarized adj transposed on PE via a regular matmul against BIG*I (scales the
mask for free).  Finalize per head: reciprocal_approx_fast on the den row,
fp16 ones-broadcast matmul, scale+bias, PE transpose out.
"""
import os
import sys
import numpy as np

for _p in ("/opt/trn_rl_repo",):
    if _p not in sys.path:
        sys.path.insert(0, _p)

B, N, D, H, F = 4, 2048, 128, 4, 32
HF = H * F           # 128
IR = 1024            # i-rows per core
NJT = N // 128       # 16 j-tiles
NCORES = 8

# engine assignment knobs
ACT_BIN = {0, 1, 3, 4, 6}          # binarize blocks on Act (sigmoid)
POOL_BIN = {2, 5, 7}               # binarize blocks on GPSIMD (is_gt)
# (h, jt) pairs whose mask-mult runs on GPSIMD instead of DVE
POOL_TT = {(h, jt) for h in range(H) for jt in range(NJT) if jt % 3 == 1}

_CACHE = {}


def build_nc(reps=1):
    import concourse.bacc as bacc
    import concourse.tile as tile
    from concourse import mybir

    f32, f16 = mybir.dt.float32, mybir.dt.float16
    Alu = mybir.AluOpType
    Act = mybir.ActivationFunctionType

    nc = bacc.Bacc(None, target_bir_lowering=False)

    xT_d   = nc.dram_tensor("xT",   [D, N],    f32, kind="ExternalInput")
    xiT_d  = nc.dram_tensor("xiT",  [D, IR],   f32, kind="ExternalInput")
    adj_d  = nc.dram_tensor("adjS", [IR, N],   f32, kind="ExternalInput")
    Wf_d   = nc.dram_tensor("Wf",   [D, HF],   f32, kind="ExternalInput")
    aS_d   = nc.dram_tensor("aS",   [HF, H],   f32, kind="ExternalInput")
    aD_d   = nc.dram_tensor("aD",   [HF, H],   f32, kind="ExternalInput")
    bias_d = nc.dram_tensor("biasR", [1, HF],  f32, kind="ExternalInput")
    out_d  = nc.dram_tensor("out",  [IR, HF],  f32, kind="ExternalOutput")

    # host constants
    EY = np.zeros((4, 4 * 128), np.float16)
    for h in range(H):
        EY[h, h * 128:(h + 1) * 128] = 1.0
    EY_d = nc.inline_tensor(EY, "EYc")
    ID16_d = nc.inline_tensor(np.eye(128, dtype=np.float16), "id16c")


    adj_r = adj_d[:].rearrange("(s p) j -> p s j", p=128)

    with tile.TileContext(nc) as tc:
        cst_ctx = tc.tile_pool(name="cst", bufs=1)
        cst = cst_ctx.__enter__()
        try:
            xT   = cst.tile([D, N], f32)
            xiT  = cst.tile([D, IR], f32)
            Wf   = cst.tile([D, HF], f32)
            aS   = cst.tile([HF, H], f32)
            aD   = cst.tile([HF, H], f32)
            biasR = cst.tile([1, HF], f32)
            biasTE = cst.tile([64, 4, 33], f16)
            eyc  = cst.tile([4, 4 * 128], f16)
            id16c = cst.tile([128, 128], f16)
            sigB = cst.tile([128, 1], f32)

            hT   = cst.tile([HF, N], f32)
            hiT  = cst.tile([HF, IR], f32)
            tAll = cst.tile([128, NJT, H], f32)
            tv1  = cst.tile([128, NJT, H], f32)   # exp(t)
            tv2  = cst.tile([128, NJT, H], f32)   # exp(.2 t)
            sZ4  = cst.tile([4, IR], f32)
            w16  = cst.tile([4, IR], f16)         # exp(.8 s) fp16
            wb16 = cst.tile([128, H, IR], f16)    # broadcast of w16 per head
            Vpack = cst.tile([128, NJT, H, 33], f16)
            mT_all = cst.tile([128, NJT, IR], f16)
            out_sb = cst.tile([128, 8, HF], f32)

            nc.sync.dma_start(Wf[:], Wf_d[:])
            nc.sync.dma_start(xiT[:], xiT_d[:])
            nc.sync.dma_start(xT[:], xT_d[:])
            nc.sync.dma_start(aS[:], aS_d[:])
            nc.sync.dma_start(aD[:], aD_d[:])
            nc.sync.dma_start(biasR[:], bias_d[:])
            nc.sync.dma_start(eyc[:], EY_d[:])
            nc.sync.dma_start(id16c[:], ID16_d[:])
            nc.vector.memset(sigB[:], -5e5)
            nc.vector.memset(biasTE[:], 0.0)
            nc.scalar.copy(
                biasTE[32:33, :, 0:32],
                biasR[:].rearrange("p (h f) -> p h f", h=H))

            # ---------------- prep ----------------
            with tc.tile_pool(name="pp", bufs=3, space="PSUM") as pp:
                # hiT = W^T x_i  (for s); hT = W^T x (for t)
                for k in range(2):
                    ps = pp.tile([HF, 512], f32, tag="pp")
                    nc.tensor.matmul(ps[:], Wf[:], xiT[:, k * 512:(k + 1) * 512],
                                     start=True, stop=True)
                    nc.scalar.copy(hiT[:, k * 512:(k + 1) * 512], ps[:])
                for k in range(2):
                    ps = pp.tile([4, 512], f32, tag="pp")
                    nc.tensor.matmul(ps[:], aS[:], hiT[:, k * 512:(k + 1) * 512],
                                     start=True, stop=True)
                    nc.scalar.copy(sZ4[:, k * 512:(k + 1) * 512], ps[:])
                nc.scalar.activation(w16[:], sZ4[:], Act.Exp, scale=0.8)
                for k in range(4):
                    ps = pp.tile([HF, 512], f32, tag="pp")
                    nc.tensor.matmul(ps[:], Wf[:], xT[:, k * 512:(k + 1) * 512],
                                     start=True, stop=True)
                    nc.scalar.copy(hT[:, k * 512:(k + 1) * 512], ps[:])
                # t per j, all heads
                for g in range(4):
                    ps = pp.tile([128, 4 * H], f32, tag="pp")
                    for k4 in range(4):
                        jt = g * 4 + k4
                        nc.tensor.matmul(ps[:, k4 * H:(k4 + 1) * H],
                                         hT[:, jt * 128:(jt + 1) * 128], aD[:],
                                         start=True, stop=True)
                    nc.scalar.copy(tAll[:, g * 4:(g + 1) * 4, :], ps[:])
                nc.scalar.activation(
                    tv1[:].rearrange("p a b -> p (a b)"),
                    tAll[:].rearrange("p a b -> p (a b)"), Act.Exp)
                nc.scalar.activation(
                    tv2[:].rearrange("p a b -> p (a b)"),
                    tAll[:].rearrange("p a b -> p (a b)"), Act.Exp, scale=0.2)
                # wb16: broadcast w16 rows to 128 partitions via PE
                for h in range(H):
                    for k in range(2):
                        ps = pp.tile([128, 512], f32, tag="pp")
                        nc.tensor.matmul(ps[:], eyc[:, h * 128:(h + 1) * 128],
                                         w16[:, k * 512:(k + 1) * 512],
                                         start=True, stop=True)
                        nc.scalar.copy(wb16[:, h, k * 512:(k + 1) * 512], ps[:])
                # Vpack: h in [j, hf] layout, fp16, with a ones column per head
                for jt in range(NJT):
                    ps = pp.tile([128, HF], f32, tag="pp")
                    nc.tensor.matmul(ps[:], xT[:, jt * 128:(jt + 1) * 128], Wf[:],
                                     start=True, stop=True)
                    nc.scalar.mul(
                        Vpack[:, jt, :, 0:32],
                        ps[:].rearrange("p (h f) -> p h f", h=H), 0.0625)
                nc.gpsimd.memset(Vpack[:, :, :, 32:33], 0.0625)

            # ---------------- main body (per rep) ----------------
            def pair_ops(gqp, pgt, h, jt):
                q = gqp.tile([128, IR], f16, tag="q")
                nc.vector.tensor_scalar(q[:], wb16[:, h, :],
                                        tv1[:, jt, h:h + 1],
                                        tv2[:, jt, h:h + 1],
                                        op0=Alu.mult, op1=Alu.max)
                pt = gqp.tile([128, IR], f16, tag="pt")
                eng = nc.gpsimd if (h, jt) in POOL_TT else nc.vector
                eng.tensor_tensor(pt[:], q[:], mT_all[:, jt, :], op=Alu.mult)
                for k in range(2):
                    nc.tensor.matmul(pgt[h][:, k * 512:(k + 1) * 512],
                                     Vpack[:, jt, h, :],
                                     pt[:, k * 512:(k + 1) * 512],
                                     start=(jt == 0), stop=(jt == NJT - 1))

            def fin(ftp, ndp, pgt, h):
                numD = ndp.tile([33, IR], f16, tag="numD")
                nc.scalar.copy(numD[:], pgt[h][:])
                tpA = ftp.tile([128, 8, 33], f32, tag="tpA")
                for c in range(8):
                    nc.tensor.matmul(tpA[:, c, :],
                                     numD[:, c * 128:(c + 1) * 128],
                                     id16c[0:33, 0:33], start=True, stop=False)
                    nc.tensor.matmul(tpA[:, c, :],
                                     numD[32:33, c * 128:(c + 1) * 128],
                                     biasTE[32:33, h, :], start=False, stop=True)
                rdT = ndp.tile([128, 8, 1], f32, tag="rdT")
                nc.vector.reciprocal_approx_fast(rdT[:], tpA[:, :, 32:33])
                nc.vector.tensor_tensor(
                    out_sb[:, :, h * 32:(h + 1) * 32], tpA[:, :, 0:32],
                    rdT[:, :, 0:1].broadcast_to([128, 8, 32]), op=Alu.mult)

            def emit_body():
                psg_ctx = tc.tile_pool(name="psg", bufs=3, space="PSUM")
                psg = psg_ctx.__enter__()
                gqp_ctx = tc.tile_pool(name="gqp", bufs=6)
                gqp = gqp_ctx.__enter__()
                pgt = {}
                pgt[0] = psg.tile([33, IR], f32, tag="pg", name="pg0")
                pgt[1] = psg.tile([33, IR], f32, tag="pg", name="pg1")
                pgt[2] = psg.tile([33, IR], f32, tag="pg", name="pg2")

                # phase 1: masks + heads 0,1 (+ head 2 lagging 2 jt)
                with tc.tile_pool(name="adjp", bufs=2) as adjp, \
                     tc.tile_pool(name="mip", bufs=2) as mip, \
                     tc.tile_pool(name="mtp", bufs=2, space="PSUM") as mtp:
                    for blk in range(8):
                        at = adjp.tile([128, 8, 256], f32, tag="adj")
                        nc.sync.dma_start(at[:], adj_r[:, :, blk * 256:(blk + 1) * 256])
                        mi = mip.tile([128, 8, 256], f16, tag="mi")
                        if blk in ACT_BIN:
                            nc.scalar.activation(
                                mi[:].rearrange("p a b -> p (a b)"),
                                at[:].rearrange("p a b -> p (a b)"),
                                Act.Sigmoid, bias=sigB[:, 0:1], scale=1e6)
                        elif blk in POOL_BIN:
                            nc.gpsimd.tensor_scalar(mi[:], at[:], 0.5, None,
                                                    op0=Alu.is_gt)
                        else:
                            nc.vector.tensor_scalar(mi[:], at[:], 0.5, None,
                                                    op0=Alu.is_gt)
                        for q in range(2):
                            jt = blk * 2 + q
                            for half in range(2):
                                pt = mtp.tile([128, 512], f32, tag="mt")
                                for s4 in range(4):
                                    s = half * 4 + s4
                                    nc.tensor.matmul(
                                        pt[:, s4 * 128:(s4 + 1) * 128],
                                        mi[:, s, q * 128:(q + 1) * 128],
                                        id16c[:], start=True, stop=True)
                                nc.scalar.copy(
                                    mT_all[:, jt, half * 512:(half + 1) * 512],
                                    pt[:])
                        for q in range(2):
                            jt = blk * 2 + q
                            pair_ops(gqp, pgt, 0, jt)
                            pair_ops(gqp, pgt, 1, jt)
                            if jt >= 2:
                                pair_ops(gqp, pgt, 2, jt - 2)

                # phase 2: finish head 2, head 3, finalizes
                ftp_ctx = tc.tile_pool(name="ftp", bufs=2, space="PSUM")
                ftp = ftp_ctx.__enter__()
                ndp_ctx = tc.tile_pool(name="ndp", bufs=4)
                ndp = ndp_ctx.__enter__()
                try:
                    pair_ops(gqp, pgt, 2, NJT - 2)
                    pair_ops(gqp, pgt, 2, NJT - 1)
                    fin(ftp, ndp, pgt, 0)
                    fin(ftp, ndp, pgt, 1)
                    pgt[3] = psg.tile([33, IR], f32, tag="pg", name="pg3")
                    for jt in range(NJT):
                        pair_ops(gqp, pgt, 3, jt)
                    fin(ftp, ndp, pgt, 2)
                    fin(ftp, ndp, pgt, 3)
                    nc.sync.dma_start(
                        out_d[:].rearrange("(s p) f -> p s f", p=128), out_sb[:])
                finally:
                    ndp_ctx.__exit__(None, None, None)
                    ftp_ctx.__exit__(None, None, None)
                    gqp_ctx.__exit__(None, None, None)
                    psg_ctx.__exit__(None, None, None)

            for _rep in range(reps):
                emit_body()
        finally:
            cst_ctx.__exit__(None, None, None)

    nc.compile()
    return nc


def _prepare_in_maps(x, adj, W, a_src, a_dst, bias):
    x = np.ascontiguousarray(np.asarray(x, dtype=np.float32))
    adj = np.asarray(adj, dtype=np.float32)
    W = np.asarray(W, dtype=np.float32)
    a_src = np.asarray(a_src, dtype=np.float32)
    a_dst = np.asarray(a_dst, dtype=np.float32)
    bias = np.asarray(bias, dtype=np.float32)

    Wf = np.ascontiguousarray(W.reshape(D, HF))
    aS = np.zeros((HF, H), np.float32)
    aD = np.zeros((HF, H), np.float32)
    for h in range(H):
        aS[h * F:(h + 1) * F, h] = a_src[h]
        aD[h * F:(h + 1) * F, h] = a_dst[h]
    biasRh = np.ascontiguousarray(bias.reshape(1, HF))

    in_maps = []
    for c in range(NCORES):
        b, cc = c // 2, c % 2
        i0 = cc * IR
        in_maps.append({
            "xT": np.ascontiguousarray(x[b].T),
            "xiT": np.ascontiguousarray(x[b, i0:i0 + IR].T),
            "adjS": np.ascontiguousarray(adj[b, i0:i0 + IR, :]),
            "Wf": Wf,
            "aS": aS,
            "aD": aD,
            "biasR": biasRh,
        })
    return in_maps


def run(inputs, trace=False, trace_cores=None):
    from concourse.bass_utils import run_bass_kernel_spmd
    if "nc" not in _CACHE:
        _CACHE["nc"] = build_nc()
    nc = _CACHE["nc"]
    in_maps = _prepare_in_maps(**inputs)
    kw = {}
    if trace:
        kw = dict(trace=True, trace_cores=trace_cores or [0])
    res = run_bass_kernel_spmd(nc, in_maps, list(range(NCORES)), **kw)
    out = np.zeros((B, N, HF), np.float32)
    for c in range(NCORES):
        b, cc = c // 2, c % 2
        out[b, cc * IR:(cc + 1) * IR, :] = res.results[c]["out"]
    return out, res


def kernel(**inputs):
    out, _ = run(inputs, trace=False)
    return out


# revision 9
# speedup vs baseline: 1.3991x; 1.0365x over previous
"""Batched GAT kernel for 8 Trainium2 NeuronCores.

Math: out[b,i,:] = softmax_j(mask(leakyrelu(s_i+t_j))) @ h  per head, concat heads.

Decomposition: exp(lrelu(e)) = max(u_i v_j, u'_i v'_j) with u=exp(s), v=exp(t),
u'=exp(.2s), v'=exp(.2t).  Dividing each row i by u'_i (cancels in softmax):
  p~_ij = m_ij * max(w_i v_j, v'_j),   w = exp(.8 s)
  out = (P~ @ [h|1]) -> num/den per head.  No G-indicator, no mask matmuls,
  no u-rescale combine: one matmul stream against a plain [h|1] fp16 pack.

Per core (c = 0..7): b = c//2, rows i in [ (c%2)*1024, +1024 ).
Per (head, j-tile): q = tensor_scalar(wb, *v_j, max v'_j) (DVE 4x) and
p~ = min(q, maskT) (DVE/GPSIMD 2x) where maskT in {0, BIG} comes from the
binarized adj transposed on PE via a regular matmul against BIG*I (scales the
mask for free).  Finalize per head: reciprocal_approx_fast on the den row,
fp16 ones-broadcast matmul, scale+bias, PE transpose out.
"""
import os
import sys
import numpy as np

for _p in ("/opt/trn_rl_repo",):
    if _p not in sys.path:
        sys.path.insert(0, _p)

B, N, D, H, F = 4, 2048, 128, 4, 32
HF = H * F           # 128
IR = 1024            # i-rows per core
NJT = N // 128       # 16 j-tiles
NCORES = 8

# engine assignment knobs
ACT_BIN = {0, 1, 3, 4, 6}          # binarize blocks on Act (sigmoid)
POOL_BIN = {2, 5, 7}               # binarize blocks on GPSIMD (is_gt)
# (h, jt) pairs whose mask-mult runs on GPSIMD instead of DVE
POOL_TT = {(h, jt) for h in range(H) for jt in range(NJT) if jt % 3 == 1}

_CACHE = {}


def build_nc(reps=1):
    import concourse.bacc as bacc
    import concourse.tile as tile
    from concourse import mybir

    f32, f16 = mybir.dt.float32, mybir.dt.float16
    Alu = mybir.AluOpType
    Act = mybir.ActivationFunctionType

    nc = bacc.Bacc(None, target_bir_lowering=False)

    xT_d   = nc.dram_tensor("xT",   [D, N],    f32, kind="ExternalInput")
    xiT_d  = nc.dram_tensor("xiT",  [D, IR],   f32, kind="ExternalInput")
    adj_d  = nc.dram_tensor("adjS", [IR, N],   f32, kind="ExternalInput")
    Wf_d   = nc.dram_tensor("Wf",   [D, HF],   f32, kind="ExternalInput")
    aS_d   = nc.dram_tensor("aS",   [HF, H],   f32, kind="ExternalInput")
    aD_d   = nc.dram_tensor("aD",   [HF, H],   f32, kind="ExternalInput")
    bias_d = nc.dram_tensor("biasR", [1, HF],  f32, kind="ExternalInput")
    out_d  = nc.dram_tensor("out",  [IR, HF],  f32, kind="ExternalOutput")

    # host constants
    EY = np.zeros((4, 4 * 128), np.float16)
    for h in range(H):
        EY[h, h * 128:(h + 1) * 128] = 1.0
    EY_d = nc.inline_tensor(EY, "EYc")
    ID16_d = nc.inline_tensor(np.eye(128, dtype=np.float16), "id16c")


    adj_r = adj_d[:].rearrange("(s p) j -> p s j", p=128)

    with tile.TileContext(nc) as tc:
        cst_ctx = tc.tile_pool(name="cst", bufs=1)
        cst = cst_ctx.__enter__()
        try:
            xT   = cst.tile([D, N], f32)
            xiT  = cst.tile([D, IR], f32)
            Wf   = cst.tile([D, HF], f32)
            aS   = cst.tile([HF, H], f32)
            aD   = cst.tile([HF, H], f32)
            biasR = cst.tile([1, HF], f32)
            biasTE = cst.tile([64, 4, 33], f16)
            eyc  = cst.tile([4, 4 * 128], f16)
            id16c = cst.tile([128, 128], f16)
            sigB = cst.tile([128, 1], f32)

            hT   = cst.tile([HF, N], f32)
            hiT  = cst.tile([HF, IR], f32)
            tAll = cst.tile([128, NJT, H], f32)
            tv1  = cst.tile([128, NJT, H], f32)   # exp(t)
            tv2  = cst.tile([128, NJT, H], f32)   # exp(.2 t)
            sZ4  = cst.tile([4, IR], f32)
            w16  = cst.tile([4, IR], f16)         # exp(.8 s) fp16
            wb16 = cst.tile([128, H, IR], f16)    # broadcast of w16 per head
            Vpack = cst.tile([128, NJT, H, 33], f16)
            mT_all = cst.tile([128, NJT, IR], f16)
            out_sb = cst.tile([128, 8, HF], f32)

            nc.sync.dma_start(Wf[:], Wf_d[:])
            nc.sync.dma_start(aS[:], aS_d[:])
            nc.sync.dma_start(aD[:], aD_d[:])
            nc.sync.dma_start(xiT[:], xiT_d[:])
            nc.sync.dma_start(eyc[:], EY_d[:])
            nc.sync.dma_start(id16c[:], ID16_d[:])
            nc.sync.dma_start(xT[:], xT_d[:])
            nc.sync.dma_start(biasR[:], bias_d[:])
            nc.vector.memset(sigB[:], -5e5)
            nc.vector.memset(biasTE[:], 0.0)
            nc.scalar.copy(
                biasTE[32:33, :, 0:32],
                biasR[:].rearrange("p (h f) -> p h f", h=H))

            # ---------------- prep ----------------
            with tc.tile_pool(name="pp", bufs=3, space="PSUM") as pp:
                # s chain first: hiT -> sZ4 -> w16 -> wb16 (feeds the TS q-ops)
                for k in range(2):
                    ps = pp.tile([HF, 512], f32, tag="pp")
                    nc.tensor.matmul(ps[:], Wf[:], xiT[:, k * 512:(k + 1) * 512],
                                     start=True, stop=True)
                    nc.scalar.copy(hiT[:, k * 512:(k + 1) * 512], ps[:])
                for k in range(2):
                    ps = pp.tile([4, 512], f32, tag="pp")
                    nc.tensor.matmul(ps[:], aS[:], hiT[:, k * 512:(k + 1) * 512],
                                     start=True, stop=True)
                    nc.scalar.copy(sZ4[:, k * 512:(k + 1) * 512], ps[:])
                nc.scalar.activation(w16[:], sZ4[:], Act.Exp, scale=0.8)
                for h in range(H):
                    for k in range(2):
                        ps = pp.tile([128, 512], f32, tag="pp")
                        nc.tensor.matmul(ps[:], eyc[:, h * 128:(h + 1) * 128],
                                         w16[:, k * 512:(k + 1) * 512],
                                         start=True, stop=True)
                        nc.scalar.copy(wb16[:, h, k * 512:(k + 1) * 512], ps[:])
                # t chain: hT -> tAll -> exps (feeds the TS scalars)
                for k in range(4):
                    ps = pp.tile([HF, 512], f32, tag="pp")
                    nc.tensor.matmul(ps[:], Wf[:], xT[:, k * 512:(k + 1) * 512],
                                     start=True, stop=True)
                    nc.scalar.copy(hT[:, k * 512:(k + 1) * 512], ps[:])
                for g in range(4):
                    ps = pp.tile([128, 4 * H], f32, tag="pp")
                    for k4 in range(4):
                        jt = g * 4 + k4
                        nc.tensor.matmul(ps[:, k4 * H:(k4 + 1) * H],
                                         hT[:, jt * 128:(jt + 1) * 128], aD[:],
                                         start=True, stop=True)
                    nc.scalar.copy(tAll[:, g * 4:(g + 1) * 4, :], ps[:])
                nc.scalar.activation(
                    tv1[:].rearrange("p a b -> p (a b)"),
                    tAll[:].rearrange("p a b -> p (a b)"), Act.Exp)
                nc.scalar.activation(
                    tv2[:].rearrange("p a b -> p (a b)"),
                    tAll[:].rearrange("p a b -> p (a b)"), Act.Exp, scale=0.2)
                # Vpack: h/16 in [j, hf] layout fp16 + 1/16 column (den headroom)
                nc.gpsimd.memset(Vpack[:, :, :, 32:33], 0.0625)
                for jt in range(NJT):
                    ps = pp.tile([128, HF], f32, tag="pp")
                    nc.tensor.matmul(ps[:], xT[:, jt * 128:(jt + 1) * 128], Wf[:],
                                     start=True, stop=True)
                    nc.scalar.mul(
                        Vpack[:, jt, :, 0:32],
                        ps[:].rearrange("p (h f) -> p h f", h=H), 0.0625)

            # ---------------- main body (per rep) ----------------
            def pair_ops(gqp, pgt, h, jt):
                q = gqp.tile([128, IR], f16, tag="q")
                nc.vector.tensor_scalar(q[:], wb16[:, h, :],
                                        tv1[:, jt, h:h + 1],
                                        tv2[:, jt, h:h + 1],
                                        op0=Alu.mult, op1=Alu.max)
                pt = gqp.tile([128, IR], f16, tag="pt")
                eng = nc.gpsimd if (h, jt) in POOL_TT else nc.vector
                eng.tensor_tensor(pt[:], q[:], mT_all[:, jt, :], op=Alu.mult)
                for k in range(2):
                    nc.tensor.matmul(pgt[h][:, k * 512:(k + 1) * 512],
                                     Vpack[:, jt, h, :],
                                     pt[:, k * 512:(k + 1) * 512],
                                     start=(jt == 0), stop=(jt == NJT - 1))

            def fin(ftp, ndp, pgt, h):
                numD = ndp.tile([33, IR], f16, tag="numD")
                nc.scalar.copy(numD[:], pgt[h][:])
                tpA = ftp.tile([128, 8, 33], f32, tag="tpA")
                for c in range(8):
                    nc.tensor.matmul(tpA[:, c, :],
                                     numD[:, c * 128:(c + 1) * 128],
                                     id16c[0:33, 0:33], start=True, stop=False)
                    nc.tensor.matmul(tpA[:, c, :],
                                     numD[32:33, c * 128:(c + 1) * 128],
                                     biasTE[32:33, h, :], start=False, stop=True)
                rdT = ndp.tile([128, 8, 1], f32, tag="rdT")
                nc.vector.reciprocal_approx_fast(rdT[:], tpA[:, :, 32:33])
                nc.vector.tensor_tensor(
                    out_sb[:, :, h * 32:(h + 1) * 32], tpA[:, :, 0:32],
                    rdT[:, :, 0:1].broadcast_to([128, 8, 32]), op=Alu.mult)

            def emit_body():
                psg_ctx = tc.tile_pool(name="psg", bufs=3, space="PSUM")
                psg = psg_ctx.__enter__()
                gqp_ctx = tc.tile_pool(name="gqp", bufs=6)
                gqp = gqp_ctx.__enter__()
                pgt = {}
                pgt[0] = psg.tile([33, IR], f32, tag="pg", name="pg0")
                pgt[1] = psg.tile([33, IR], f32, tag="pg", name="pg1")
                pgt[2] = psg.tile([33, IR], f32, tag="pg", name="pg2")

                # phase 1: masks + heads 0,1 (+ head 2 lagging 2 jt)
                with tc.tile_pool(name="adjp", bufs=2) as adjp, \
                     tc.tile_pool(name="mip", bufs=2) as mip, \
                     tc.tile_pool(name="mtp", bufs=2, space="PSUM") as mtp:
                    for blk in range(8):
                        at = adjp.tile([128, 8, 256], f32, tag="adj")
                        nc.sync.dma_start(at[:], adj_r[:, :, blk * 256:(blk + 1) * 256])
                        mi = mip.tile([128, 8, 256], f16, tag="mi")
                        if blk in ACT_BIN:
                            nc.scalar.activation(
                                mi[:].rearrange("p a b -> p (a b)"),
                                at[:].rearrange("p a b -> p (a b)"),
                                Act.Sigmoid, bias=sigB[:, 0:1], scale=1e6)
                        elif blk in POOL_BIN:
                            nc.gpsimd.tensor_scalar(mi[:], at[:], 0.5, None,
                                                    op0=Alu.is_gt)
                        else:
                            nc.vector.tensor_scalar(mi[:], at[:], 0.5, None,
                                                    op0=Alu.is_gt)
                        for q in range(2):
                            jt = blk * 2 + q
                            for half in range(2):
                                pt = mtp.tile([128, 512], f32, tag="mt")
                                for s4 in range(4):
                                    s = half * 4 + s4
                                    nc.tensor.matmul(
                                        pt[:, s4 * 128:(s4 + 1) * 128],
                                        mi[:, s, q * 128:(q + 1) * 128],
                                        id16c[:], start=True, stop=True)
                                nc.scalar.copy(
                                    mT_all[:, jt, half * 512:(half + 1) * 512],
                                    pt[:])
                        for q in range(2):
                            jt = blk * 2 + q
                            pair_ops(gqp, pgt, 0, jt)
                            pair_ops(gqp, pgt, 1, jt)
                            if jt >= 2:
                                pair_ops(gqp, pgt, 2, jt - 2)

                # phase 2: finish head 2, head 3, finalizes
                ftp_ctx = tc.tile_pool(name="ftp", bufs=2, space="PSUM")
                ftp = ftp_ctx.__enter__()
                ndp_ctx = tc.tile_pool(name="ndp", bufs=4)
                ndp = ndp_ctx.__enter__()
                try:
                    pair_ops(gqp, pgt, 2, NJT - 2)
                    pair_ops(gqp, pgt, 2, NJT - 1)
                    fin(ftp, ndp, pgt, 0)
                    pgt[3] = psg.tile([33, IR], f32, tag="pg", name="pg3")
                    for jt in range(NJT):
                        pair_ops(gqp, pgt, 3, jt)
                        if jt == 3:
                            fin(ftp, ndp, pgt, 1)
                        elif jt == 7:
                            fin(ftp, ndp, pgt, 2)
                    fin(ftp, ndp, pgt, 3)
                    nc.sync.dma_start(
                        out_d[:].rearrange("(s p) f -> p s f", p=128), out_sb[:])
                finally:
                    ndp_ctx.__exit__(None, None, None)
                    ftp_ctx.__exit__(None, None, None)
                    gqp_ctx.__exit__(None, None, None)
                    psg_ctx.__exit__(None, None, None)

            for _rep in range(reps):
                emit_body()
        finally:
            cst_ctx.__exit__(None, None, None)

    nc.compile()
    return nc


def _prepare_in_maps(x, adj, W, a_src, a_dst, bias):
    x = np.ascontiguousarray(np.asarray(x, dtype=np.float32))
    adj = np.asarray(adj, dtype=np.float32)
    W = np.asarray(W, dtype=np.float32)
    a_src = np.asarray(a_src, dtype=np.float32)
    a_dst = np.asarray(a_dst, dtype=np.float32)
    bias = np.asarray(bias, dtype=np.float32)

    Wf = np.ascontiguousarray(W.reshape(D, HF))
    aS = np.zeros((HF, H), np.float32)
    aD = np.zeros((HF, H), np.float32)
    for h in range(H):
        aS[h * F:(h + 1) * F, h] = a_src[h]
        aD[h * F:(h + 1) * F, h] = a_dst[h]
    biasRh = np.ascontiguousarray(bias.reshape(1, HF))

    in_maps = []
    for c in range(NCORES):
        b, cc = c // 2, c % 2
        i0 = cc * IR
        in_maps.append({
            "xT": np.ascontiguousarray(x[b].T),
            "xiT": np.ascontiguousarray(x[b, i0:i0 + IR].T),
            "adjS": np.ascontiguousarray(adj[b, i0:i0 + IR, :]),
            "Wf": Wf,
            "aS": aS,
            "aD": aD,
            "biasR": biasRh,
        })
    return in_maps


def run(inputs, trace=False, trace_cores=None):
    from concourse.bass_utils import run_bass_kernel_spmd
    if "nc" not in _CACHE:
        _CACHE["nc"] = build_nc()
    nc = _CACHE["nc"]
    in_maps = _prepare_in_maps(**inputs)
    kw = {}
    if trace:
        kw = dict(trace=True, trace_cores=trace_cores or [0])
    res = run_bass_kernel_spmd(nc, in_maps, list(range(NCORES)), **kw)
    out = np.zeros((B, N, HF), np.float32)
    for c in range(NCORES):
        b, cc = c // 2, c % 2
        out[b, cc * IR:(cc + 1) * IR, :] = res.results[c]["out"]
    return out, res


def kernel(**inputs):
    out, _ = run(inputs, trace=False)
    return out


# revision 11
# speedup vs baseline: 1.5741x; 1.1251x over previous
"""Batched GAT kernel for 8 Trainium2 NeuronCores.

Math: out[b,i,:] = softmax_j(mask(leakyrelu(s_i+t_j))) @ h  per head, concat heads.

Decomposition: exp(lrelu(e)) = max(u_i v_j, u'_i v'_j) with u=exp(s), v=exp(t),
u'=exp(.2s), v'=exp(.2t).  Dividing each row i by u'_i (cancels in softmax):
  p~_ij = m_ij * max(w_i v_j, v'_j),   w = exp(.8 s)
  out = (P~ @ [h|1]) -> num/den per head.  No G-indicator, no mask matmuls,
  no u-rescale combine: one matmul stream against a plain [h|1] fp16 pack.

Per core (c = 0..7): b = c//2, rows i in [ (c%2)*1024, +1024 ).
Per (head, j-tile): q = tensor_scalar(wb, *v_j, max v'_j) (DVE 4x) and
p~ = min(q, maskT) (DVE/GPSIMD 2x) where maskT in {0, BIG} comes from the
binarized adj transposed on PE via a regular matmul against BIG*I (scales the
mask for free).  Finalize per head: reciprocal_approx_fast on the den row,
fp16 ones-broadcast matmul, scale+bias, PE transpose out.
"""
import os
import sys
import numpy as np

for _p in ("/opt/trn_rl_repo",):
    if _p not in sys.path:
        sys.path.insert(0, _p)

B, N, D, H, F = 4, 2048, 128, 4, 32
HF = H * F           # 128
IR = 1024            # i-rows per core
NJT = N // 128       # 16 j-tiles
NCORES = 8

# engine assignment knobs
ACT_BIN = {0, 1, 3, 4, 6}          # binarize blocks on Act (sigmoid)
POOL_BIN = {2, 5, 7}               # binarize blocks on GPSIMD (is_gt)
# (h, jt) pairs whose mask-mult runs on GPSIMD instead of DVE
POOL_TT = {(h, jt) for h in range(H) for jt in range(NJT) if jt % 4 == 2}

_CACHE = {}


def build_nc(reps=1):
    import concourse.bacc as bacc
    import concourse.tile as tile
    from concourse import mybir

    f32, f16 = mybir.dt.float32, mybir.dt.float16
    Alu = mybir.AluOpType
    Act = mybir.ActivationFunctionType

    nc = bacc.Bacc(None, target_bir_lowering=False)

    xT_d   = nc.dram_tensor("xT",   [D, N],    f32, kind="ExternalInput")
    xiT_d  = nc.dram_tensor("xiT",  [D, IR],   f32, kind="ExternalInput")
    adj_d  = nc.dram_tensor("adjS", [IR, N],   f32, kind="ExternalInput")
    Wf_d   = nc.dram_tensor("Wf",   [D, HF],   f32, kind="ExternalInput")
    aS_d   = nc.dram_tensor("aS",   [HF, H],   f32, kind="ExternalInput")
    aD_d   = nc.dram_tensor("aD",   [HF, H],   f32, kind="ExternalInput")
    bias_d = nc.dram_tensor("biasR", [1, HF],  f32, kind="ExternalInput")
    out_d  = nc.dram_tensor("out",  [IR, HF],  f32, kind="ExternalOutput")

    # host constants
    EY = np.zeros((4, 4 * 128), np.float16)
    for h in range(H):
        EY[h, h * 128:(h + 1) * 128] = 1.0
    EY_d = nc.inline_tensor(EY, "EYc")
    ID16_d = nc.inline_tensor(np.eye(128, dtype=np.float16), "id16c")


    adj_r = adj_d[:].rearrange("(s p) j -> p s j", p=128)

    with tile.TileContext(nc) as tc:
        cst_ctx = tc.tile_pool(name="cst", bufs=1)
        cst = cst_ctx.__enter__()
        try:
            xT   = cst.tile([D, N], f32)
            xiT  = cst.tile([D, IR], f32)
            Wf   = cst.tile([D, HF], f32)
            aS   = cst.tile([HF, H], f32)
            aD   = cst.tile([HF, H], f32)
            biasR = cst.tile([1, HF], f32)
            biasTE = cst.tile([64, 4, 33], f16)
            eyc  = cst.tile([4, 4 * 128], f16)
            id16c = cst.tile([128, 128], f16)
            sigB = cst.tile([128, 1], f32)

            Wf16 = cst.tile([D, HF], f16)
            aS16 = cst.tile([HF, H], f16)
            aD16 = cst.tile([HF, H], f16)
            xT16 = cst.tile([D, N], f16)
            xiT16 = cst.tile([D, IR], f16)
            hT16 = cst.tile([HF, N], f16)
            hiT16 = cst.tile([HF, IR], f16)
            tAll = cst.tile([128, NJT, H], f32)
            tv1  = cst.tile([128, NJT, H], f32)   # exp(t)
            tv2  = cst.tile([128, NJT, H], f32)   # exp(.2 t)
            sZ4  = cst.tile([4, IR], f32)
            w16  = cst.tile([4, IR], f16)         # exp(.8 s) fp16
            wb16 = cst.tile([128, H, IR], f16)    # broadcast of w16 per head
            Vpack = cst.tile([128, NJT, H, 33], f16)
            mT_all = cst.tile([128, NJT, IR], f16)
            out_sb = cst.tile([128, 8, HF], f32)

            nc.sync.dma_start(Wf[:], Wf_d[:])
            nc.sync.dma_start(aS[:], aS_d[:])
            nc.sync.dma_start(aD[:], aD_d[:])
            nc.sync.dma_start(xiT[:], xiT_d[:])
            nc.sync.dma_start(eyc[:], EY_d[:])
            nc.sync.dma_start(id16c[:], ID16_d[:])
            nc.sync.dma_start(xT[:], xT_d[:])
            nc.sync.dma_start(biasR[:], bias_d[:])
            nc.vector.memset(sigB[:], -5e5)
            nc.vector.memset(biasTE[:], 0.0)
            nc.scalar.copy(
                biasTE[32:33, :, 0:32],
                biasR[:].rearrange("p (h f) -> p h f", h=H))

            # ---------------- prep ----------------
            nc.scalar.copy(Wf16[:], Wf[:])
            nc.scalar.copy(aS16[:], aS[:])
            nc.scalar.copy(aD16[:], aD[:])
            nc.scalar.copy(xiT16[:], xiT[:])
            nc.scalar.copy(xT16[:], xT[:])
            with tc.tile_pool(name="pp", bufs=3, space="PSUM") as pp:
                # s chain first: hiT -> sZ4 -> w16 -> wb16 (feeds the TS q-ops)
                for k in range(2):
                    ps = pp.tile([HF, 512], f32, tag="pp")
                    nc.tensor.matmul(ps[:], Wf16[:], xiT16[:, k * 512:(k + 1) * 512],
                                     start=True, stop=True)
                    nc.vector.tensor_copy(hiT16[:, k * 512:(k + 1) * 512], ps[:])
                for k in range(2):
                    ps = pp.tile([4, 512], f32, tag="pp")
                    nc.tensor.matmul(ps[:], aS16[:], hiT16[:, k * 512:(k + 1) * 512],
                                     start=True, stop=True)
                    nc.scalar.copy(sZ4[:, k * 512:(k + 1) * 512], ps[:])
                nc.scalar.activation(w16[:], sZ4[:], Act.Exp, scale=0.8)
                for h in range(H):
                    for k in range(2):
                        ps = pp.tile([128, 512], f32, tag="pp")
                        nc.tensor.matmul(ps[:], eyc[:, h * 128:(h + 1) * 128],
                                         w16[:, k * 512:(k + 1) * 512],
                                         start=True, stop=True)
                        nc.vector.tensor_copy(wb16[:, h, k * 512:(k + 1) * 512], ps[:])
                # t chain: hT -> tAll -> exps (feeds the TS scalars)
                for k in range(4):
                    ps = pp.tile([HF, 512], f32, tag="pp")
                    nc.tensor.matmul(ps[:], Wf16[:], xT16[:, k * 512:(k + 1) * 512],
                                     start=True, stop=True)
                    nc.vector.tensor_copy(hT16[:, k * 512:(k + 1) * 512], ps[:])
                for g in range(4):
                    ps = pp.tile([128, 4 * H], f32, tag="pp")
                    for k4 in range(4):
                        jt = g * 4 + k4
                        nc.tensor.matmul(ps[:, k4 * H:(k4 + 1) * H],
                                         hT16[:, jt * 128:(jt + 1) * 128], aD16[:],
                                         start=True, stop=True)
                    nc.scalar.copy(tAll[:, g * 4:(g + 1) * 4, :], ps[:])
                nc.scalar.activation(
                    tv1[:].rearrange("p a b -> p (a b)"),
                    tAll[:].rearrange("p a b -> p (a b)"), Act.Exp)
                nc.scalar.activation(
                    tv2[:].rearrange("p a b -> p (a b)"),
                    tAll[:].rearrange("p a b -> p (a b)"), Act.Exp, scale=0.2)
                # Vpack: h/16 in [j, hf] layout fp16 + 1/16 column (den headroom)
                nc.gpsimd.memset(Vpack[:, :, :, 32:33], 0.0625)
                for jt in range(NJT):
                    ps = pp.tile([128, HF], f32, tag="pp")
                    nc.tensor.matmul(ps[:], xT16[:, jt * 128:(jt + 1) * 128], Wf16[:],
                                     start=True, stop=True)
                    nc.vector.tensor_scalar(
                        Vpack[:, jt, :, 0:32],
                        ps[:].rearrange("p (h f) -> p h f", h=H),
                        0.0625, None, op0=Alu.mult)

            # ---------------- main body (per rep) ----------------
            def pair_ops(gqp, pgt, h, jt):
                q = gqp.tile([128, IR], f16, tag="q")
                nc.vector.tensor_scalar(q[:], wb16[:, h, :],
                                        tv1[:, jt, h:h + 1],
                                        tv2[:, jt, h:h + 1],
                                        op0=Alu.mult, op1=Alu.max)
                pt = gqp.tile([128, IR], f16, tag="pt")
                eng = nc.gpsimd if (h, jt) in POOL_TT else nc.vector
                eng.tensor_tensor(pt[:], q[:], mT_all[:, jt, :], op=Alu.mult)
                for k in range(2):
                    nc.tensor.matmul(pgt[h][:, k * 512:(k + 1) * 512],
                                     Vpack[:, jt, h, :],
                                     pt[:, k * 512:(k + 1) * 512],
                                     start=(jt == 0), stop=(jt == NJT - 1))

            def fin(ftp, ndp, pgt, h):
                numD = ndp.tile([33, IR], f16, tag="numD")
                nc.scalar.copy(numD[:], pgt[h][:])
                tpA = ftp.tile([128, 8, 33], f32, tag="tpA")
                for c in range(8):
                    nc.tensor.matmul(tpA[:, c, :],
                                     numD[:, c * 128:(c + 1) * 128],
                                     id16c[0:33, 0:33], start=True, stop=False)
                    nc.tensor.matmul(tpA[:, c, :],
                                     numD[32:33, c * 128:(c + 1) * 128],
                                     biasTE[32:33, h, :], start=False, stop=True)
                rdT = ndp.tile([128, 8, 1], f32, tag="rdT")
                nc.vector.reciprocal_approx_fast(rdT[:], tpA[:, :, 32:33])
                nc.vector.tensor_tensor(
                    out_sb[:, :, h * 32:(h + 1) * 32], tpA[:, :, 0:32],
                    rdT[:, :, 0:1].broadcast_to([128, 8, 32]), op=Alu.mult)

            def emit_body():
                psg_ctx = tc.tile_pool(name="psg", bufs=3, space="PSUM")
                psg = psg_ctx.__enter__()
                gqp_ctx = tc.tile_pool(name="gqp", bufs=6)
                gqp = gqp_ctx.__enter__()
                pgt = {}
                pgt[0] = psg.tile([33, IR], f32, tag="pg", name="pg0")
                pgt[1] = psg.tile([33, IR], f32, tag="pg", name="pg1")
                pgt[2] = psg.tile([33, IR], f32, tag="pg", name="pg2")

                # phase 1: masks + heads 0,1 (+ head 2 lagging 2 jt)
                with tc.tile_pool(name="adjp", bufs=2) as adjp, \
                     tc.tile_pool(name="mip", bufs=2) as mip, \
                     tc.tile_pool(name="mtp", bufs=2, space="PSUM") as mtp:
                    for blk in range(8):
                        at = adjp.tile([128, 8, 256], f32, tag="adj")
                        nc.sync.dma_start(at[:], adj_r[:, :, blk * 256:(blk + 1) * 256])
                        mi = mip.tile([128, 8, 256], f16, tag="mi")
                        if blk in ACT_BIN:
                            nc.scalar.activation(
                                mi[:].rearrange("p a b -> p (a b)"),
                                at[:].rearrange("p a b -> p (a b)"),
                                Act.Sigmoid, bias=sigB[:, 0:1], scale=1e6)
                        elif blk in POOL_BIN:
                            nc.gpsimd.tensor_scalar(mi[:], at[:], 0.5, None,
                                                    op0=Alu.is_gt)
                        else:
                            nc.vector.tensor_scalar(mi[:], at[:], 0.5, None,
                                                    op0=Alu.is_gt)
                        for q in range(2):
                            jt = blk * 2 + q
                            for half in range(2):
                                pt = mtp.tile([128, 512], f32, tag="mt")
                                for s4 in range(4):
                                    s = half * 4 + s4
                                    nc.tensor.matmul(
                                        pt[:, s4 * 128:(s4 + 1) * 128],
                                        mi[:, s, q * 128:(q + 1) * 128],
                                        id16c[:], start=True, stop=True)
                                nc.scalar.copy(
                                    mT_all[:, jt, half * 512:(half + 1) * 512],
                                    pt[:])
                        for q in range(2):
                            jt = blk * 2 + q
                            pair_ops(gqp, pgt, 0, jt)
                            pair_ops(gqp, pgt, 1, jt)
                            if jt >= 2:
                                pair_ops(gqp, pgt, 2, jt - 2)

                # phase 2: finish head 2, head 3, finalizes
                ftp_ctx = tc.tile_pool(name="ftp", bufs=2, space="PSUM")
                ftp = ftp_ctx.__enter__()
                ndp_ctx = tc.tile_pool(name="ndp", bufs=4)
                ndp = ndp_ctx.__enter__()
                try:
                    pair_ops(gqp, pgt, 2, NJT - 2)
                    pair_ops(gqp, pgt, 2, NJT - 1)
                    fin(ftp, ndp, pgt, 0)
                    pgt[3] = psg.tile([33, IR], f32, tag="pg", name="pg3")
                    for jt in range(NJT):
                        pair_ops(gqp, pgt, 3, jt)
                        if jt == 3:
                            fin(ftp, ndp, pgt, 1)
                        elif jt == 7:
                            fin(ftp, ndp, pgt, 2)
                    fin(ftp, ndp, pgt, 3)
                    nc.sync.dma_start(
                        out_d[:].rearrange("(s p) f -> p s f", p=128), out_sb[:])
                finally:
                    ndp_ctx.__exit__(None, None, None)
                    ftp_ctx.__exit__(None, None, None)
                    gqp_ctx.__exit__(None, None, None)
                    psg_ctx.__exit__(None, None, None)

            for _rep in range(reps):
                emit_body()
        finally:
            cst_ctx.__exit__(None, None, None)

    nc.compile()
    return nc


def _prepare_in_maps(x, adj, W, a_src, a_dst, bias):
    x = np.ascontiguousarray(np.asarray(x, dtype=np.float32))
    adj = np.asarray(adj, dtype=np.float32)
    W = np.asarray(W, dtype=np.float32)
    a_src = np.asarray(a_src, dtype=np.float32)
    a_dst = np.asarray(a_dst, dtype=np.float32)
    bias = np.asarray(bias, dtype=np.float32)

    Wf = np.ascontiguousarray(W.reshape(D, HF))
    aS = np.zeros((HF, H), np.float32)
    aD = np.zeros((HF, H), np.float32)
    for h in range(H):
        aS[h * F:(h + 1) * F, h] = a_src[h]
        aD[h * F:(h + 1) * F, h] = a_dst[h]
    biasRh = np.ascontiguousarray(bias.reshape(1, HF))

    in_maps = []
    for c in range(NCORES):
        b, cc = c // 2, c % 2
        i0 = cc * IR
        in_maps.append({
            "xT": np.ascontiguousarray(x[b].T),
            "xiT": np.ascontiguousarray(x[b, i0:i0 + IR].T),
            "adjS": np.ascontiguousarray(adj[b, i0:i0 + IR, :]),
            "Wf": Wf,
            "aS": aS,
            "aD": aD,
            "biasR": biasRh,
        })
    return in_maps


def run(inputs, trace=False, trace_cores=None):
    from concourse.bass_utils import run_bass_kernel_spmd
    if "nc" not in _CACHE:
        _CACHE["nc"] = build_nc()
    nc = _CACHE["nc"]
    in_maps = _prepare_in_maps(**inputs)
    kw = {}
    if trace:
        kw = dict(trace=True, trace_cores=trace_cores or [0])
    res = run_bass_kernel_spmd(nc, in_maps, list(range(NCORES)), **kw)
    out = np.zeros((B, N, HF), np.float32)
    for c in range(NCORES):
        b, cc = c // 2, c % 2
        out[b, cc * IR:(cc + 1) * IR, :] = res.results[c]["out"]
    return out, res


def kernel(**inputs):
    out, _ = run(inputs, trace=False)
    return out
